# revision 6
# baseline (speedup 1.0000x reference)
"""Trainium2 Bass kernel for the CoOccurrenceEncoder pairwise-MLP problem.

Reference computation (per batch b of 4, N=512 nodes, d=128):
    hi = x @ W1[:d]          # [N, d]
    hj = x @ W1[d:]          # [N, d]
    h  = relu(hi[:,None,:] + hj[None,:,:] + b1)       # [N, N, d]
    h2 = relu(h @ W2 + b2)                            # [N, N, 64]
    out = sigmoid((h2 @ W3 + b3)[..., 0])             # [N, N]

Sharding: 8 cores; core c handles batch c//2, i-rows [256*(c%2), 256*(c%2)+256).
Each core holds its batch's full hj [d=128 partitions, N=512] in SBUF (bf16) and
streams 256 i-rows; weights are tiny and replicated.

Per-core dataflow (d on partitions everywhere):
  stage1 (DVE, bf16 4x): S_i = relu(hj + (hi_i + b1))  via one tensor_scalar
          (per-partition vector add + max-with-0) per row -> SBUF bf16 [128,512]
  stage2 (PE): lhsT = [W2 | W2] (128x128 bf16 stationary); two i-rows run as
          column-tiled matmuls into one PSUM bank pair -> h2 fp32 [128,1024]
  stage2b (ACT/DVE split): relu(h2 + b2) PSUM -> SBUF bf16
  stage3 (PE): lhsT strip s holds [W3;0],[0;W3] at cols 2s -> logits [2,512]
          written at partition 32s of a shared PSUM pair (8 pairs per 2 banks)
  stage4 (ACT): sigmoid(logits + b3) -> SBUF fp32, DMA (HWDGE) to HBM.
"""

import numpy as np
import ml_dtypes

import concourse.bass as bass
import concourse.mybir as mybir
import concourse.tile as tile
from concourse import bacc
from concourse.bass_utils import run_bass_kernel_spmd

F32 = mybir.dt.float32
BF16 = mybir.dt.bfloat16

D = 128          # feature dim (= partitions)
N = 512          # nodes per batch
B = 4            # batches
NCORES = 8
ROWS = 256       # i-rows per core
PAIRS = ROWS // 2
NQ = PAIRS // 2  # loop iterations, 2 pairs (4 rows) each

# every DVE_RELU_PERIOD-th q does its stage2-relu on DVE instead of ACT
DVE_RELU_PERIOD = 4


def build_nc():
    # Bacc (not plain Bass): its compile() runs move_matmul_waits_to_ldweights
    # + generate_event_semaphores, needed to satisfy TRN2's 1-wait-per-matmul
    # hardware constraint.
    nc = bacc.Bacc("TRN2")

    xT_d = nc.dram_tensor("xT", [D, N], BF16, kind="ExternalInput")
    xTi_d = nc.dram_tensor("xTi", [D, ROWS], BF16, kind="ExternalInput")
    w1a_d = nc.dram_tensor("w1a", [D, D], BF16, kind="ExternalInput")
    w1b_d = nc.dram_tensor("w1b", [D, D], BF16, kind="ExternalInput")
    w2d_d = nc.dram_tensor("w2d", [D, D], BF16, kind="ExternalInput")
    t3_d = nc.dram_tensor("t3", [D, 8], BF16, kind="ExternalInput")
    b1_d = nc.dram_tensor("b1c", [D, 1], F32, kind="ExternalInput")
    b2_d = nc.dram_tensor("b2c", [D, 1], F32, kind="ExternalInput")
    b3_d = nc.dram_tensor("b3c", [D, 1], F32, kind="ExternalInput")
    out_d = nc.dram_tensor("out", [ROWS, N], F32, kind="ExternalOutput")

    AT = mybir.ActivationFunctionType
    OP = mybir.AluOpType

    with tile.TileContext(nc) as tc:
        with tc.tile_pool(name="singles", bufs=1) as singles:
            xt = singles.tile([D, N], BF16)
            xti = singles.tile([D, ROWS], BF16)
            w1a = singles.tile([D, D], BF16)
            w1b = singles.tile([D, D], BF16)
            w2d = singles.tile([D, D], BF16)
            t3 = singles.tile([D, 8], BF16)
            b1 = singles.tile([D, 1], F32)
            b2 = singles.tile([D, 1], F32)
            b3 = singles.tile([D, 1], F32)
            hjsb = singles.tile([D, N], BF16)
            bias = singles.tile([D, ROWS], F32)

            nc.sync.dma_start(xt[:], xT_d[:])
            nc.sync.dma_start(xti[:], xTi_d[:])
            nc.sync.dma_start(w1a[:], w1a_d[:])
            nc.sync.dma_start(w1b[:], w1b_d[:])
            nc.sync.dma_start(w2d[:], w2d_d[:])
            nc.sync.dma_start(t3[:], t3_d[:])
            nc.sync.dma_start(b1[:], b1_d[:])
            nc.sync.dma_start(b2[:], b2_d[:])
            nc.sync.dma_start(b3[:], b3_d[:])

            # ---- prep: hj (bf16) and per-row bias = hi + b1 (fp32) ----
            with tc.tile_pool(name="prep_ps", bufs=1, space="PSUM") as pps:
                hj_ps = pps.tile([D, N], F32)
                nc.tensor.matmul(hj_ps[:], lhsT=w1b[:], rhs=xt[:])
                nc.vector.tensor_copy(hjsb[:], hj_ps[:])

                hi_ps = pps.tile([D, ROWS], F32)
                nc.tensor.matmul(hi_ps[:], lhsT=w1a[:], rhs=xti[:])
                nc.vector.tensor_scalar(
                    bias[:], hi_ps[:], b1[:, 0:1], None, OP.add
                )

            # ---- main loop: 2 pairs (4 rows) per q ----
            with (
                tc.tile_pool(name="spool", bufs=6) as spool,
                tc.tile_pool(name="h2pool", bufs=3) as h2pool,
                tc.tile_pool(name="opool", bufs=2) as opool,
                tc.tile_pool(name="ps2pool", bufs=2, space="PSUM") as ps2pool,
                tc.tile_pool(name="ps3pool", bufs=2, space="PSUM") as ps3pool,
            ):
                ps3 = None
                for q in range(NQ):
                    rows = [4 * q + r for r in range(4)]

                    ss = []
                    for r, i in enumerate(rows):
                        s = spool.tile([D, N], BF16, tag="s")
                        nc.vector.tensor_scalar(
                            s[:], hjsb[:], bias[:, i:i + 1], 0.0, OP.add, OP.max
                        )
                        ss.append(s)

                    ps2 = ps2pool.tile([D, 2 * N], F32)
                    nc.tensor.matmul(ps2[0:64, 0:N], lhsT=w2d[:, 0:64], rhs=ss[0][:])
                    nc.tensor.matmul(ps2[64:128, 0:N], lhsT=w2d[:, 64:128], rhs=ss[1][:])
                    nc.tensor.matmul(ps2[0:64, N:2 * N], lhsT=w2d[:, 0:64], rhs=ss[2][:])
                    nc.tensor.matmul(ps2[64:128, N:2 * N], lhsT=w2d[:, 64:128], rhs=ss[3][:])

                    h2r = h2pool.tile([D, 2 * N], BF16, tag="h2r")
                    if q % DVE_RELU_PERIOD == 0:
                        nc.vector.tensor_scalar(
                            h2r[:], ps2[:], b2[:, 0:1], 0.0, OP.add, OP.max
                        )
                    else:
                        nc.scalar.activation(
                            h2r[:], ps2[:], AT.Relu, bias=b2[:, 0:1], scale=1.0
                        )

                    # stage 3: pair p -> slot sl = p % 8, bank = sl//4, strip = sl%4
                    if q % 4 == 0:
                        ps3 = ps3pool.tile([D, 2 * N], F32)
                    for a in range(2):  # two pairs in this q
                        p = 2 * q + a
                        sl = p % 8
                        bank, s = sl // 4, sl % 4
                        nc.tensor.matmul(
                            ps3[32 * s:32 * s + 2, N * bank:N * bank + N],
                            lhsT=t3[:, 2 * s:2 * s + 2],
                            rhs=h2r[:, N * a:N * a + N],
                            tile_position=(0, 32 * s),
                        )

                    if q % 4 == 3:
                        k = q // 4  # group of 8 pairs = 16 rows
                        sig = opool.tile([D, 2 * N], F32, tag="sig")
                        nc.scalar.activation(
                            sig[:], ps3[:], AT.Sigmoid, bias=b3[:, 0:1], scale=1.0
                        )
                        # partition 32*s + r, half b  <->  out row 16k + 8b + 2s + r
                        # one DMA per strip s: src has a single contiguous
                        # partition dim (2 partitions); multi-dim partition
                        # APs mis-lower (second dim read as in-partition
                        # offset).
                        dst16 = out_d[16 * k:16 * k + 16, :].rearrange(
                            "(b s2 r) j -> s2 r b j", b=2, s2=4
                        )
                        for s in range(4):
                            src = sig[32 * s:32 * s + 2, :].rearrange(
                                "p (b j) -> p b j", b=2
                            )
                            nc.sync.dma_start(dst16[s], src)
    nc.finalize()
    return nc


_CACHED_NC = None


def _get_nc():
    global _CACHED_NC
    if _CACHED_NC is None:
        _CACHED_NC = build_nc()
    return _CACHED_NC


def _host_prep(node_features, W1, b1, W2, b2, W3, b3):
    bf = ml_dtypes.bfloat16
    w1a = np.ascontiguousarray(W1[:D].astype(bf))
    w1b = np.ascontiguousarray(W1[D:].astype(bf))
    w2d = np.ascontiguousarray(np.concatenate([W2, W2], axis=1).astype(bf))
    t3 = np.zeros((D, 8), np.float32)
    for s in range(4):
        t3[0:64, 2 * s] = W3[:, 0]
        t3[64:128, 2 * s + 1] = W3[:, 0]
    t3 = t3.astype(bf)
    b1c = np.ascontiguousarray(b1.reshape(D, 1).astype(np.float32))
    b2c = np.ascontiguousarray(np.concatenate([b2, b2]).reshape(D, 1).astype(np.float32))
    b3c = np.full((D, 1), np.float32(b3[0]), np.float32)

    in_maps = []
    for c in range(NCORES):
        b, half = c // 2, c % 2
        xT = np.ascontiguousarray(node_features[b].T.astype(bf))
        xTi = np.ascontiguousarray(xT[:, half * ROWS:(half + 1) * ROWS])
        in_maps.append({
            "xT": xT, "xTi": xTi,
            "w1a": w1a, "w1b": w1b, "w2d": w2d, "t3": t3,
            "b1c": b1c, "b2c": b2c, "b3c": b3c,
        })
    return in_maps


def run(node_features, W1, b1, W2, b2, W3, b3, **spmd_kwargs):
    """Run the bass kernel; returns (full_output, BassKernelResults)."""
    nc = _get_nc()
    in_maps = _host_prep(node_features, W1, b1, W2, b2, W3, b3)
    res = run_bass_kernel_spmd(nc, in_maps, core_ids=list(range(NCORES)), **spmd_kwargs)
    out = np.empty((B, N, N), np.float32)
    for c in range(NCORES):
        b, half = c // 2, c % 2
        out[b, half * ROWS:(half + 1) * ROWS, :] = res.results[c]["out"]
    return out, res


def kernel(node_features, W1, b1, W2, b2, W3, b3):
    out, _ = run(node_features, W1, b1, W2, b2, W3, b3)
    return out
